# revision 13
# baseline (speedup 1.0000x reference)
"""Trainium2 Bass kernel for nn_MultiGat (2-layer GAT + mean-pool + MLP).

Strategy (8 NeuronCores, SPMD single program):
  - Nodes are sharded 2500/core (padded to 2560).  Each core owns the edges
    whose *destination* lands in its node range, sorted by destination and
    grouped per 128-node destination tile (padded to G groups of 128 edges).
  - Per layer, a node "table" row [h(256) | alpha_src(4) | alpha_dst(4) | pad]
    (320 f32 = 1280 B) lives in DRAM per core (layer 1 computed redundantly;
    layer 2 distributed via AllGather of per-core shards).
  - Edge phase per core, per destination tile: dma_gather of src rows
    (1280 B) + dst alpha rows (256 B), per-edge p = exp(leaky_relu(.)) and
    messages [p*h | p] on DVE/ACT, then aggregation on the TensorEngine:
    for each group of 128 edges, a one-hot edge->dst matrix B (built on DVE
    by comparing the edge's tile-local dst id against an iota row; padding
    edges use -1 so their column is all-zero) and matmul accumulation
    out[128 dst, 260] += B^T @ msg in PSUM.  PSUM handles duplicate
    destinations natively - no scatter-add, no races.
  - Softmax uses exp without max-subtraction (values are O(1); the per-node
    normalizer cancels), with the reference's +1e-16 in the denominator.
  - Biases are folded: b is added to h pre-aggregation (softmax weights sum
    to 1), and the alpha contributions of b are host-folded constants.
  - Mean-pool partials per core via a one-hot (1/cnt) matmul; host sums the
    8 partials and runs the tiny 256->128->10 MLP in numpy.
"""

import sys

sys.path.insert(0, "/opt/trn_rl_repo")

import numpy as np


# ----------------------------------------------------------------------------
# configuration
# ----------------------------------------------------------------------------
def full_cfg():
    return dict(
        PC=8,          # cores
        NG=20000,      # global nodes
        NLOC=2500,     # nodes per core
        NPAD=2560,     # padded nodes per core (multiple of 128)
        H=4, C=64, HC=256,
        ROW=320,       # table row width (f32): h(256) as(4) ad(4) pad(56)
        G=19,          # edge groups (of 128) per destination tile
        B=32,          # graphs
        FIN=64,        # input features (pos 2 + x 62)
    )


def mini_cfg():
    return dict(
        PC=8, NG=512, NLOC=64, NPAD=128, H=4, C=64, HC=256, ROW=320,
        G=4, B=4, FIN=64,
    )


# ----------------------------------------------------------------------------
# device program
# ----------------------------------------------------------------------------

def _patch_tile_swdge_lane_by_queue():
    """Pin each Pool-engine DMA instruction's DMASW sem lane to its SWDGE
    queue_num (Tile's default round-robin mixes queues on one sem lane,
    which the scheduler rejects when num_swdge_queues > 1)."""
    import concourse.tile_sem_assignment as tsa
    if getattr(tsa, "_lane_by_queue_patched", False):
        return
    tsa._lane_by_queue_patched = True
    import concourse.mybir as mybir
    import concourse.bass_isa as bass_isa

    orig = tsa.TileClockTick._assign_tick

    def _assign_tick(self, inst):
        from concourse.tile_scheduler import DMAInst
        if (
            isinstance(inst, DMAInst)
            and not isinstance(inst, bass_isa.UserSyncedRemoteDMADescs)
            and inst.engine == mybir.EngineType.Pool
        ):
            q = int(getattr(inst, "queue_num", 0) or 0)
            self.next_sw_dma_idx = q
        return orig(self, inst)

    tsa.TileClockTick._assign_tick = _assign_tick


def build_program(cfg, reps=1):
    import concourse.mybir as mybir
    import concourse.bacc as bacc
    import concourse.tile as tile

    f32 = mybir.dt.float32
    i16 = mybir.dt.int16
    AF = mybir.ActivationFunctionType

    PC, NPAD, ROW, HC, H, C, G = (
        cfg["PC"], cfg["NPAD"], cfg["ROW"], cfg["HC"], cfg["H"], cfg["C"],
        cfg["G"])
    B, FIN = cfg["B"], cfg["FIN"]
    NTBL = PC * NPAD           # table rows (global, padded)
    NT_T = NTBL // 128         # node tiles for table build
    NL_T = NPAD // 128         # local node tiles (= edge chunks per layer)
    CH = G * 128               # tokens per chunk (one dst tile)
    EPAD = CH * NL_T           # padded edge tokens per core
    ICOLS = EPAD // 16
    NAUG = HC + H              # aggregated row: [msg(256) | p(4)]

    _patch_tile_swdge_lane_by_queue()
    NQ = cfg.get("NQ", 4)
    nc = bacc.Bacc(None, target_bir_lowering=False, debug=True,
                   num_swdge_queues=NQ)

    # ---- I/O
    xt = nc.declare_dram_parameter("xt", [FIN, NTBL], f32, isOutput=False)
    w1 = nc.declare_dram_parameter("w1", [FIN, ROW], f32, isOutput=False)
    b1f = nc.declare_dram_parameter("b1f", [128, ROW], f32, isOutput=False)
    w2 = nc.declare_dram_parameter("w2", [128, 2, ROW], f32, isOutput=False)
    b2f = nc.declare_dram_parameter("b2f", [128, ROW], f32, isOutput=False)
    ident = nc.declare_dram_parameter("ident", [128, 128], f32, isOutput=False)
    iota = nc.declare_dram_parameter("iota", [128, 128], f32, isOutput=False)
    srcw = nc.declare_dram_parameter("srcw", [128, ICOLS], i16, isOutput=False)
    gdstw = nc.declare_dram_parameter("gdstw", [128, ICOLS], i16, isOutput=False)
    bloc = nc.declare_dram_parameter("bloc", [128, NL_T * G], f32, isOutput=False)
    mpool = nc.declare_dram_parameter("mpool", [128, NL_T, B], f32, isOutput=False)
    pooled = nc.declare_dram_parameter("pooled", [B, HC], f32, isOutput=True)

    # ---- internal DRAM
    T1 = nc.dram_tensor("T1", [NTBL, ROW], f32)
    T2s = nc.dram_tensor("T2s", [NPAD, ROW], f32)
    T2 = nc.dram_tensor("T2", [NTBL, ROW], f32, addr_space="Shared")

    with tile.TileContext(nc) as tc:
        with tc.tile_pool(name="persist", bufs=1) as pp:
            si = pp.tile([128, ICOLS], i16)
            gi = pp.tile([128, ICOLS], i16)
            bl = pp.tile([128, NL_T * G], f32)
            io = pp.tile([128, 128], f32)
            nc.sync.dma_start(si[:], srcw[:])
            nc.sync.dma_start(gi[:], gdstw[:])
            nc.sync.dma_start(bl[:], bloc[:])
            nc.sync.dma_start(io[:], iota[:])

            for _rep in range(reps):
                # ------------- phase 0: build T1 (replicated) -------------
                with (
                    tc.tile_pool(name="p0", bufs=3) as p0,
                    tc.tile_pool(name="p0w", bufs=1) as p0w,
                    tc.tile_pool(name="ps0", bufs=4, space="PSUM") as ps0,
                ):
                    xts = p0w.tile([FIN, NTBL], f32)
                    nc.sync.dma_start(xts[:], xt[:])
                    w1s = p0w.tile([FIN, ROW], f32)
                    nc.sync.dma_start(w1s[:], w1[:])
                    b1s = p0w.tile([128, ROW], f32)
                    nc.sync.dma_start(b1s[:], b1f[:])
                    for j in range(NT_T):
                        j0 = j * 128
                        ps = ps0.tile([128, ROW], f32)
                        nc.tensor.matmul(ps[:], xts[:, j0:j0 + 128], w1s[:],
                                         start=True, stop=True)
                        ts = p0.tile([128, ROW], f32)
                        nc.vector.tensor_add(ts[:], ps[:], b1s[:])
                        nc.sync.dma_start(T1[j0:j0 + 128, :], ts[:])

                # ------------- edge phase: one dst tile per chunk ----------
                # consume(j, o) receives the normalized output tile
                # o [128, HC] for local node tile j.
                def edge_phase(T, tag, consume):
                    if cfg.get("SKIP_EDGES"):
                        for k in range(NL_T):
                            with tc.tile_pool(name=f"z{tag}", bufs=1) as zp:
                                o = zp.tile([128, HC], f32)
                                nc.vector.memset(o[:], 0.0)
                                consume(k, o)
                        return
                    with (
                        tc.tile_pool(name=f"e{tag}", bufs=2) as ep,
                        tc.tile_pool(name=f"es{tag}", bufs=3) as esp,
                        tc.tile_pool(name=f"eps{tag}", bufs=2,
                                     space="PSUM") as epsp,
                    ):
                        for k in range(NL_T):
                            cols = slice(k * (CH // 16), (k + 1) * (CH // 16))
                            g1 = ep.tile([128, G, ROW], f32, tag="g1")
                            nc.gpsimd.dma_gather(
                                g1[:], T[:, :], si[:, cols], CH, CH, ROW,
                                elem_step=ROW, single_packet=False,
                                queue_num=k % 2 if NQ > 1 else 0)
                            g2 = ep.tile([128, G, 64], f32, tag="g2")
                            nc.gpsimd.dma_gather(
                                g2[:], T[:, HC:HC + 64], gi[:, cols], CH, CH,
                                64, elem_step=ROW, single_packet=False,
                                queue_num=(2 + k % 2) if NQ > 1 else 0)
                            se = esp.tile([128, G, H], f32, tag="se")
                            nc.vector.tensor_add(
                                se[:], g1[:, :, HC:HC + H], g2[:, :, H:2 * H])
                            lr = esp.tile([128, G, H], f32, tag="lr")
                            nc.vector.tensor_scalar_mul(lr[:], se[:], 0.2)
                            lr2 = esp.tile([128, G, H], f32, tag="lr2")
                            nc.vector.tensor_max(lr2[:], se[:], lr[:])
                            mp = ep.tile([128, G, NAUG], f32, tag="mp")
                            nc.scalar.activation(
                                mp[:, :, HC:HC + H], lr2[:], AF.Exp)
                            pv = mp[:, :, HC:HC + H]
                            pb = pv.unsqueeze(3).broadcast_to([128, G, H, C])
                            nc.vector.tensor_mul(
                                mp[:, :, 0:HC].rearrange(
                                    "p m (h c) -> p m h c", c=C),
                                g1[:, :, 0:HC].rearrange(
                                    "p m (h c) -> p m h c", c=C),
                                pb)
                            # aggregate via one-hot matmuls
                            acc = epsp.tile([128, NAUG], f32, tag="acc")
                            for g in range(G):
                                bt = esp.tile([128, 128], f32, tag="bt")
                                nc.vector.tensor_scalar(
                                    bt[:], io[:],
                                    bl[:, k * G + g:k * G + g + 1],
                                    None, mybir.AluOpType.is_equal)
                                nc.tensor.matmul(
                                    acc[:], bt[:], mp[:, g, :],
                                    start=(g == 0), stop=(g == G - 1))
                            # normalize: o = num / (den + 1e-16)
                            nc.vector.tensor_scalar_add(
                                acc[:, HC:HC + H], acc[:, HC:HC + H], 1e-16)
                            rd = esp.tile([128, H], f32, tag="rd")
                            nc.vector.reciprocal(rd[:], acc[:, HC:HC + H])
                            o = esp.tile([128, HC], f32, tag="o")
                            for h in range(H):
                                nc.vector.tensor_scalar_mul(
                                    o[:, h * C:(h + 1) * C],
                                    acc[:, h * C:(h + 1) * C], rd[:, h:h + 1])
                            consume(k, o)

                # ------------- layer 1 + transpose into o1T -------------
                with (
                    tc.tile_pool(name="p2w", bufs=1) as p2w,
                    tc.tile_pool(name="pst", bufs=4, space="PSUM") as pst,
                ):
                    ids = p2w.tile([128, 128], f32)
                    nc.sync.dma_start(ids[:], ident[:])
                    o1T = p2w.tile([128, 2, NPAD], f32)

                    def consume1(j, o):
                        j0 = j * 128
                        for kk in range(2):
                            pt = pst.tile([128, 128], f32, tag="pt")
                            nc.tensor.transpose(
                                pt[:], o[:, kk * 128:(kk + 1) * 128], ids[:])
                            nc.vector.tensor_copy(
                                o1T[:, kk, j0:j0 + 128], pt[:])

                    edge_phase(T1, "1", consume1)

                    # ------------- T2 shard + AllGather -------------
                    with (
                        tc.tile_pool(name="p2", bufs=3) as p2,
                        tc.tile_pool(name="ps2", bufs=4, space="PSUM") as ps2,
                    ):
                        w2s = p2w.tile([128, 2, ROW], f32)
                        nc.sync.dma_start(w2s[:], w2[:])
                        b2s = p2w.tile([128, ROW], f32)
                        nc.sync.dma_start(b2s[:], b2f[:])
                        for j in range(NL_T):
                            j0 = j * 128
                            ps = ps2.tile([128, ROW], f32, tag="mm")
                            nc.tensor.matmul(ps[:], o1T[:, 0, j0:j0 + 128],
                                             w2s[:, 0, :],
                                             start=True, stop=False)
                            nc.tensor.matmul(ps[:], o1T[:, 1, j0:j0 + 128],
                                             w2s[:, 1, :],
                                             start=False, stop=True)
                            ts = p2.tile([128, ROW], f32, tag="t2row")
                            nc.vector.tensor_add(ts[:], ps[:], b2s[:])
                            nc.sync.dma_start(T2s[j0:j0 + 128, :], ts[:])

                        nc.gpsimd.collective_compute(
                            "AllGather",
                            mybir.AluOpType.bypass,
                            replica_groups=[list(range(PC))],
                            ins=[T2s[:]],
                            outs=[T2[:]],
                        )

                # ------------- layer 2 + pooling -------------
                with (
                    tc.tile_pool(name="p4w", bufs=1) as p4w,
                    tc.tile_pool(name="ps4", bufs=2, space="PSUM") as ps4,
                ):
                    o2buf = p4w.tile([128, NL_T, HC], f32)

                    def consume2(j, o):
                        nc.vector.tensor_copy(o2buf[:, j, :], o[:])

                    edge_phase(T2, "2", consume2)

                    mps = p4w.tile([128, NL_T, B], f32)
                    nc.sync.dma_start(mps[:], mpool[:])
                    acc = ps4.tile([B, HC], f32)
                    for j in range(NL_T):
                        nc.tensor.matmul(acc[:], mps[:, j, :], o2buf[:, j, :],
                                         start=(j == 0), stop=(j == NL_T - 1))
                    po = p4w.tile([B, HC], f32)
                    nc.vector.tensor_copy(po[:], acc[:])
                    nc.sync.dma_start(pooled[:], po[:])

        _, _snap = tc.schedule_and_allocate()
        nc.predicted_ns = _snap.time if _snap is not None else None

    nc.compile()
    return nc


# ----------------------------------------------------------------------------
# host-side preparation
# ----------------------------------------------------------------------------
def pack_edges(cfg, src_g, dst_g, core):
    """Sort this core's edges by destination, group per 128-node dst tile,
    pad each tile's run to G*128 tokens.  Returns (src_idx, gdst_idx, bloc)
    where bloc[t] is the tile-local dst id (0..127) or -1 for padding."""
    NLOC, NPAD, G = cfg["NLOC"], cfg["NPAD"], cfg["G"]
    NL_T = NPAD // 128
    CH = G * 128
    EPAD = CH * NL_T
    lo = core * NLOC
    sel = (dst_g >= lo) & (dst_g < lo + NLOC)
    es = src_g[sel]
    ed = dst_g[sel] - lo
    order = np.argsort(ed, kind="stable")
    es, ed = es[order], ed[order]

    src_idx = np.zeros(EPAD, dtype=np.int16)
    gdst_idx = np.zeros(EPAD, dtype=np.int16)
    bloc = np.full(EPAD, -1.0, dtype=np.float32)
    remap = lambda gidx: (gidx // NLOC) * NPAD + (gidx % NLOC)
    tile_of = ed // 128
    starts = np.searchsorted(tile_of, np.arange(NL_T), side="left")
    ends = np.searchsorted(tile_of, np.arange(NL_T), side="right")
    for t in range(NL_T):
        a, b = starts[t], ends[t]
        cnt = b - a
        assert cnt <= CH, f"dst tile {t} has {cnt} edges > capacity {CH}"
        p0 = t * CH
        src_idx[p0:p0 + cnt] = remap(es[a:b]).astype(np.int16)
        gdst_idx[p0:p0 + cnt] = remap(ed[a:b] + lo).astype(np.int16)
        bloc[p0:p0 + cnt] = (ed[a:b] - t * 128).astype(np.float32)
    return src_idx, gdst_idx, bloc


def wrap16(idx):
    """[EPAD] token array -> [128, EPAD/16] wrapped+replicated layout."""
    w = idx.reshape(-1, 16).T  # [16, EPAD/16]
    return np.ascontiguousarray(np.tile(w, (8, 1)))


def wrap128(vals):
    """[EPAD] token array -> [128, EPAD/128] (token t at [t%128, t//128])."""
    return np.ascontiguousarray(vals.reshape(-1, 128).T)


def host_prepare(cfg, x, pos, edge_index, batch,
                 W1, a_src1, a_dst1, b1, W2, a_src2, a_dst2, b2):
    PC, NG, NLOC, NPAD, H, C, HC, FIN, B = (
        cfg["PC"], cfg["NG"], cfg["NLOC"], cfg["NPAD"], cfg["H"], cfg["C"],
        cfg["HC"], cfg["FIN"], cfg["B"])
    NTBL = PC * NPAD

    x_in = np.concatenate([pos, x], axis=1).astype(np.float32)  # [NG, FIN]
    loop = np.arange(NG, dtype=np.int64)
    src = np.concatenate([np.asarray(edge_index[0]), loop])
    dst = np.concatenate([np.asarray(edge_index[1]), loop])

    xpad = np.zeros((NTBL, FIN), np.float32)
    for c in range(PC):
        xpad[c * NPAD:c * NPAD + NLOC] = x_in[c * NLOC:(c + 1) * NLOC]
    xt = np.ascontiguousarray(xpad.T)

    def augment(W, a_s, a_d, b):
        ROW = cfg["ROW"]
        wad = np.einsum("fhc,hc->fh", W.reshape(W.shape[0], H, C), a_d)
        was = np.einsum("fhc,hc->fh", W.reshape(W.shape[0], H, C), a_s)
        waug = np.concatenate(
            [W, was, wad,
             np.zeros((W.shape[0], ROW - HC - 2 * H), np.float32)],
            axis=1).astype(np.float32)
        cs = np.einsum("hc,hc->h", b.reshape(H, C), a_s)
        cd = np.einsum("hc,hc->h", b.reshape(H, C), a_d)
        brow = np.concatenate(
            [b, cs, cd,
             np.zeros(ROW - HC - 2 * H, np.float32)]).astype(np.float32)
        return waug, brow

    w1aug, b1row = augment(W1, a_src1, a_dst1, b1)
    w2aug, b2row = augment(W2, a_src2, a_dst2, b2)
    b1f = np.ascontiguousarray(np.broadcast_to(b1row, (128, b1row.shape[0])))
    b2f = np.ascontiguousarray(np.broadcast_to(b2row, (128, b2row.shape[0])))
    w2k = np.ascontiguousarray(
        w2aug.reshape(2, 128, w2aug.shape[1]).transpose(1, 0, 2))
    ident = np.eye(128, dtype=np.float32)
    iota = np.ascontiguousarray(
        np.broadcast_to(np.arange(128, dtype=np.float32), (128, 128)))

    cnt = np.bincount(np.asarray(batch).astype(np.int64), minlength=B)
    in_maps = []
    for c in range(PC):
        si, gi, blv = pack_edges(cfg, src, dst, c)
        mp = np.zeros((NPAD, B), np.float32)
        gb = np.asarray(batch)[c * NLOC:(c + 1) * NLOC].astype(np.int64)
        mp[np.arange(NLOC), gb] = 1.0 / np.maximum(cnt[gb], 1.0)
        mpool = np.ascontiguousarray(
            mp.reshape(NPAD // 128, 128, B).transpose(1, 0, 2))
        in_maps.append(dict(
            xt=xt, w1=w1aug, b1f=b1f, w2=w2k, b2f=b2f, ident=ident, iota=iota,
            srcw=wrap16(si), gdstw=wrap16(gi), bloc=wrap128(blv), mpool=mpool,
        ))
    return in_maps


def host_tail(pooled_parts, lw1, lb1, lw2, lb2):
    pooled = np.sum(np.stack(pooled_parts), axis=0)
    y = np.maximum(pooled @ lw1 + lb1, 0.0)
    y = np.maximum(y @ lw2 + lb2, 0.0)
    return y.astype(np.float32)


# ----------------------------------------------------------------------------
# entry point
# ----------------------------------------------------------------------------
_CACHE = {}


def kernel(**inputs):
    from concourse.bass_utils import run_bass_kernel_spmd

    cfg = full_cfg()
    inp = {k: np.asarray(v) for k, v in inputs.items()}
    in_maps = host_prepare(
        cfg, inp["x"], inp["pos"], inp["edge_index"], inp["batch"],
        inp["W1"], inp["a_src1"], inp["a_dst1"], inp["b1"],
        inp["W2"], inp["a_src2"], inp["a_dst2"], inp["b2"])
    if "nc" not in _CACHE:
        _CACHE["nc"] = build_program(cfg)
    nc = _CACHE["nc"]
    res = run_bass_kernel_spmd(nc, in_maps, list(range(cfg["PC"])))
    parts = [res.results[c]["pooled"] for c in range(cfg["PC"])]
    return host_tail(parts, inp["lw1"], inp["lb1"], inp["lw2"], inp["lb2"])
